# revision 32
# baseline (speedup 1.0000x reference)
# NonLocalBlock Trainium2 Bass kernel.
#
# Reference computation (per batch b):
#   theta = theta_w @ X + theta_b          [IC, N]   (X = x[b] as [C, N])
#   phi   = phi_w   @ X + phi_b            [IC, N]
#   g     = g_w     @ X + g_b              [IC, N]
#   attn  = softmax_j(theta^T phi)         [N, N]
#   att   = g @ attn^T                     [IC, N]
#   y     = BN(w_w @ att + w_b) + x
#
# Math folds used on device (validated vs reference):
#   - phi bias drops out of softmax entirely (adds an i-only constant).
#   - g bias folds into the final bias because attn rows sum to 1.
#   - BN is affine: fold into w_eff = inv*w_w and b_final.
#   - scores bounded (|s| < 50) so exp() needs no max-subtraction.
#
# Sharding: 8 cores = 4 batches x 2 row-halves. Each core receives x[b]
# with its own half's columns swapped to the front, so every core runs an
# identical program (pure SPMD).
#
# v3 design (from HW traces of v1 @133us, v2 @119us):
#   - ACT exp() is the hard floor (~66us busy at [128,1536] groups); the
#     whole schedule exists to keep it saturated from ~10us on.
#   - All 44 (block x j-group) score groups are emitted inside the 8-slice
#     x-load loop, interleaved across blocks in (group, block) order: one
#     phi slice unlocks scores for every ready block at once, so the ACT
#     backlog absorbs any PE/DVE jitter (v2 lost ~25us in phase 1 to an
#     in-order DVE queue ping-ponging with ACT).
#   - Softmax denominator: exp emits bf16; per-block partial sums run as
#     TWO independent accumulator chains - groups 0-4 on the DVE, groups
#     5-10 on the otherwise-idle GPSIMD - merged on GPSIMD, then one
#     3x512-moving ones-matmul per block folds across partitions. This
#     keeps the DVE (~19us of unavoidable 1x-rate PSUM-source proj
#     copies) well under the ACT floor.
#   - Residual+bias fused into one scalar_tensor_tensor per store:
#     yo = (y_psum + b_final[k]) + x  (kills the xb precompute adds).
#   - PSUM: sc 2x[128,1536] (6 banks) + att 1 + tail 1. Phase-1
#     projections batch th|ph|g into one sc-sized slot per slice;
#     g-transposes batch 4 chunks into the tail bank.
#   - AV consumes drain block-major (att accumulates in a single PSUM
#     bank per block); on a block's last AV, its tail (den-fold, recip,
#     normalize, W, store) is emitted inline and hides under the
#     ACT-bound stream.

from collections import deque
from contextlib import ExitStack

import numpy as np

import concourse.bass as bass
import concourse.tile as tile
from concourse import bacc, mybir
from concourse.bass_utils import run_bass_kernel_spmd

F32 = mybir.dt.float32
F32R = mybir.dt.float32r
BF16 = mybir.dt.bfloat16
AF = mybir.ActivationFunctionType
ADD = mybir.AluOpType.add

B, C, IC = 4, 256, 128
H = W = 64
N = H * W            # 4096
HALF = N // 2        # 2048 rows of attention per core
P = 128
NCORES = 8
NBLK = HALF // 512   # 4 i-blocks of 512
NCH = N // P         # 32 j-chunks of 128
# groups of j-chunks per block: 10x3 + 1x2 = 32 chunks
GRPS = [list(range(3 * g, 3 * g + 3)) for g in range(10)] + [[30, 31]]
NGRP = len(GRPS)     # 11
DEFER = 2            # consume exp output this many groups late
NWARM = 6            # HAM warmup matmuls at t=0 (512-col)
EXBUFS = 20
BN_EPS = 1e-5


def _r(ap):
    return ap.bitcast(F32R)


class _Pipe:
    """Emission state for the flat (group, block)-interleaved pipeline."""

    def __init__(self, nc, pools, yout):
        self.nc = nc
        self.p = pools
        self.yout = yout
        self.av_q = [deque() for _ in range(NBLK)]
        self.av_blk = 0      # avs drain block-major
        self.gt_ready = 0    # j-chunks whose gT transpose has been emitted
        self.att = {}
        self.accd = {}
        self.exs = {}
        self.emit_idx = {}

    # ---- scores + exp + denominator chain links ------------------------
    def emit_group(self, blk, g):
        nc, p = self.nc, self.p
        chunks = GRPS[g]
        w = 512 * len(chunks)
        isl = slice(blk * 512, (blk + 1) * 512)
        sc = p["ps"].tile([P, 1536], F32, name=f"sc{blk}_{g}", tag="sc",
                          bufs=2)
        for pos, c in enumerate(chunks):
            nc.tensor.matmul(
                sc[:, pos * 512:(pos + 1) * 512],
                p["phi_sb"][:, c * P:(c + 1) * P],
                p["theta_sb"][:, isl],
                start=True, stop=True)
        ex = p["ex"].tile([P, 1536], BF16, name=f"ex{blk}_{g}", tag="ex",
                          bufs=EXBUFS)
        nc.scalar.activation(ex[:, :w], sc[:, :w], AF.Exp)
        self.exs[(blk, g)] = ex
        # denominator partial-sum chain (DVE, bf16 2x rate). Block 0's
        # chain is NOT emitted here: its groups emit during the x-load
        # loop where the DVE is busy with projection copies - the caller
        # bursts its chain right after the loop instead.
        if blk != 0:
            self.den_add(blk, g)
        self.av_q[blk].append((blk, g, ex))

    def den_add(self, blk, g):
        nc, p = self.nc, self.p
        ex = self.exs[(blk, g)]
        w = 512 * len(GRPS[g])
        if g == 0:
            return
        if g == 1:
            self.accd[blk] = p["rec"].tile(
                [P, 1536], BF16, name=f"acd{blk}", tag="accd", bufs=2)
            nc.vector.tensor_add(self.accd[blk][:],
                                 self.exs[(blk, 0)][:], ex[:])
        else:
            nc.vector.tensor_add(self.accd[blk][:, :w],
                                 self.accd[blk][:, :w], ex[:, :w])

    # ---- AV consume (g @ attn^T numerator) -----------------------------
    def _emit_av(self, blk, g, ex):
        nc, p = self.nc, self.p
        if g == 0:
            self.att[blk] = p["ps"].tile([P, 512], F32, name=f"att{blk}",
                                         tag="att", bufs=1)
        for pos, c in enumerate(GRPS[g]):
            nc.tensor.matmul(
                self.att[blk][:],
                p["gT_sb"][:, c * P:(c + 1) * P],
                ex[:, pos * 512:(pos + 1) * 512],
                start=c == 0, stop=c == NCH - 1)
        if g == NGRP - 1:
            self._emit_tail(blk)
            self.av_blk += 1

    def drain_av(self, emitted, all_=False):
        """Drain pending avs block-major; an av is eligible once its exp
        was emitted DEFER group-emissions ago AND its gT chunks' transposes
        have been emitted (emission order IS dependency order - a read
        emitted before its writer silently reads stale SBUF). At most 2
        per call so a backlog never opens a multi-us PE gap in the score
        stream."""
        n = 0
        while self.av_blk < NBLK and (all_ or n < 3):
            q = self.av_q[self.av_blk]
            if not q:
                break
            blk, g, ex = q[0]
            if GRPS[g][-1] >= self.gt_ready:
                break
            idx = self.emit_idx[(blk, g)]
            if not all_ and idx > emitted - DEFER:
                break
            q.popleft()
            self._emit_av(blk, g, ex)
            n += 1

    # ---- block tail: den-fold, recip, normalize, W, store --------------
    def _emit_tail(self, blk):
        nc, p = self.nc, self.p
        acc = self.accd[blk]
        den = p["ps"].tile([P, 512], F32, name=f"den{blk}", tag="tail",
                           bufs=1)
        for s in range(3):
            nc.tensor.matmul(den[:], p["onesb"][:],
                             acc[:, s * 512:(s + 1) * 512],
                             start=s == 0, stop=s == 2)
        recb = p["rec"].tile([P, 512], F32, name=f"recb{blk}", tag="recb",
                             bufs=2)
        nc.vector.reciprocal_approx_fast(out=recb[:], in_=den[:])
        attn = p["rec"].tile([P, 512], F32R, name=f"attn{blk}", tag="attn",
                             bufs=2)
        nc.vector.tensor_mul(attn[:], self.att[blk][:], recb[:])
        isl = slice(blk * 512, (blk + 1) * 512)
        for k in range(2):
            y = p["ps"].tile([P, 512], F32, name=f"y{blk}_{k}", tag="tail",
                             bufs=1)
            nc.tensor.matmul(y[:], p["wef_sb"][:, k * P:(k + 1) * P],
                             attn[:], start=True, stop=True)
            yo = p["rec"].tile([P, 512], F32, name=f"yo{blk}_{k}", tag="yo",
                               bufs=2)
            nc.vector.scalar_tensor_tensor(
                out=yo[:], in0=y[:], scalar=p["bfin_sb"][:, k:k + 1],
                in1=p["x_sb"][k][:, isl], op0=ADD, op1=ADD)
            nc.sync.dma_start(out=self.yout[k * P:(k + 1) * P, isl],
                              in_=yo[:])


def _kernel_body(ctx, tc, ins, yout):
    nc = tc.nc
    xin, thw, phw, gw, wef, tb, bfin = (
        ins["xin"], ins["thw"], ins["phw"], ins["gw"], ins["wef"],
        ins["tb"], ins["bfin"])

    consts = ctx.enter_context(tc.tile_pool(name="consts", bufs=1))
    big = ctx.enter_context(tc.tile_pool(name="big", bufs=1))

    # ---- dummies; exdum goes first on the ACT queue so walrus's
    # auto-inserted exp table load lands in the preamble window.
    dum_r = consts.tile([P, 512], F32R, name="dum_r")
    nc.vector.memset(dum_r.bitcast(F32)[:], 1.0)
    exdum = consts.tile([P, 1], F32, name="exdum")
    nc.scalar.activation(exdum[:], dum_r.bitcast(F32)[:, 0:1], AF.Exp)

    # ---- x load: 512-col slices, both HWDGE rings in parallel ----
    x_sb = [big.tile([P, N], F32R, name=f"x_sb{k}") for k in range(2)]

    # k=1 ring rides the GpSimd queue: the Scalar queue must stay
    # exp-only (a DMA descriptor issue costs ~650ns of ACT queue time
    # and serializes ahead of the exps).
    def xdma(t, k):
        tsl = slice(t * 512, (t + 1) * 512)
        eng = nc.sync if k == 0 else nc.gpsimd
        eng.dma_start(out=x_sb[k][:, tsl],
                      in_=_r(xin[k * P:(k + 1) * P, tsl]))

    # sync ring carries ONLY the x k=0 stream (plus the output stores
    # later); everything else rides gpsimd so slice 0 is never queued
    # behind weight transfers.
    for k in range(2):
        xdma(0, k)
    for t in range(1, 8):
        xdma(t, 0)
    thw_sb = consts.tile([P, C], F32R, name="thw_sb")
    phw_sb = consts.tile([P, C], F32R, name="phw_sb")
    gw_sb = consts.tile([P, C], F32R, name="gw_sb")
    for k in range(2):
        nc.gpsimd.dma_start(out=thw_sb[:, k * P:(k + 1) * P],
                            in_=_r(thw[k * P:(k + 1) * P, :]))
        nc.gpsimd.dma_start(out=phw_sb[:, k * P:(k + 1) * P],
                            in_=_r(phw[k * P:(k + 1) * P, :]))
        nc.gpsimd.dma_start(out=gw_sb[:, k * P:(k + 1) * P],
                            in_=_r(gw[k * P:(k + 1) * P, :]))
    tb_sb = consts.tile([P, 1], F32, name="tb_sb")
    xdma(1, 1)
    nc.gpsimd.dma_start(out=tb_sb[:], in_=tb[:, None])
    xdma(2, 1)
    ident = consts.tile([P, P], F32, name="ident")
    nc.gpsimd.dma_start(out=ident[:], in_=ins["ident"][:, :])
    for t in range(3, 8):
        xdma(t, 1)
    wef_sb = consts.tile([P, C], F32R, name="wef_sb")
    nc.gpsimd.dma_start(out=wef_sb[:], in_=_r(wef[:, :]))
    bfin_sb = consts.tile([P, 2], F32, name="bfin_sb")
    nc.gpsimd.dma_start(out=bfin_sb[:],
                        in_=bfin.rearrange("(k p) -> p k", p=P))
    onesb = consts.tile([P, P], BF16, name="onesb")
    nc.vector.memset(onesb[:], 1.0)

    theta_sb = big.tile([P, HALF], F32R, name="theta_sb")
    phi_sb = big.tile([P, N], F32R, name="phi_sb")
    g_sb = big.tile([P, N], F32, name="g_sb")
    gT_sb = big.tile([P, N], BF16, name="gT_sb")

    # ---- PSUM: sc 2x[128,1536]=6 banks, att 1, tail 1 ----
    ps_pool = ctx.enter_context(tc.tile_pool(name="ps", bufs=1, space="PSUM"))
    pools = {
        "ps": ps_pool,
        "ex": ctx.enter_context(tc.tile_pool(name="ex", bufs=EXBUFS)),
        "rec": ctx.enter_context(tc.tile_pool(name="rec", bufs=2)),
        "theta_sb": theta_sb, "phi_sb": phi_sb, "gT_sb": gT_sb,
        "onesb": onesb, "wef_sb": wef_sb, "bfin_sb": bfin_sb, "x_sb": x_sb,
    }
    pipe = _Pipe(nc, pools, yout)

    # ---- HAM warmup while the first x slice streams in ----
    dum_ps = ps_pool.tile([P, 512], F32, name="dum_ps", tag="tail", bufs=1)
    for i in range(NWARM):
        nc.tensor.matmul(dum_ps[:], dum_r[:, 0:P], dum_r[:],
                         start=True, stop=True)

    # ---- per-slice projections batched into one sc-sized PSUM slot
    # (th | ph | g); g-transposes batch 4 chunks into the tail bank.
    def proj(t):
        tsl = slice(t * 512, (t + 1) * 512)
        slot = ps_pool.tile([P, 1536], F32, name=f"proj{t}", tag="sc",
                            bufs=2)
        if t < NBLK:
            for k in range(2):
                nc.tensor.matmul(slot[:, 0:512],
                                 thw_sb[:, k * P:(k + 1) * P],
                                 x_sb[k][:, tsl],
                                 start=k == 0, stop=k == 1)
        for k in range(2):
            nc.tensor.matmul(slot[:, 512:1024],
                             phw_sb[:, k * P:(k + 1) * P],
                             x_sb[k][:, tsl],
                             start=k == 0, stop=k == 1)
        for k in range(2):
            nc.tensor.matmul(slot[:, 1024:1536],
                             gw_sb[:, k * P:(k + 1) * P],
                             x_sb[k][:, tsl],
                             start=k == 0, stop=k == 1)
        # phi first: it is the copy that gates the next score groups
        nc.vector.tensor_copy(phi_sb[:, tsl], slot[:, 512:1024])
        if t < NBLK:
            nc.vector.tensor_scalar_add(theta_sb[:, tsl], slot[:, 0:512],
                                        tb_sb[:])
        nc.vector.tensor_copy(g_sb[:, tsl], slot[:, 1024:1536])

    def transposes(t):
        pst = ps_pool.tile([P, 512], F32, name=f"gt{t}", tag="tail", bufs=1)
        for j in range(4):
            jc = 4 * t + j
            nc.tensor.transpose(pst[:, j * P:(j + 1) * P],
                                g_sb[:, jc * P:(jc + 1) * P], ident[:])
        nc.vector.tensor_copy(gT_sb[:, 4 * t * P:(4 * t + 4) * P], pst[:])
        pipe.gt_ready = 4 * (t + 1)

    # group g needs phi slice (last_chunk // 4); block blk needs theta
    # slice blk. In the slice loop emit at most TWO ready groups per slice
    # (block-major priority) - the proj slot then rotates onto a slot
    # whose last reader is cheap, so projections track the DMA stream
    # instead of slipping to exp pace. The remaining groups bulk-emit
    # after the loop, block-major, paced by the sc slots.
    t_req = [GRPS[g][-1] // 4 for g in range(NGRP)]
    emitted = 0

    def emit_one(blk, g):
        nonlocal emitted
        pipe.emit_idx[(blk, g)] = emitted
        pipe.emit_group(blk, g)
        emitted += 1
        pipe.drain_av(emitted)

    # Projections run ONE SLICE AHEAD of their score groups: a proj's sc
    # slot predecessor (2 allocations back in the rotation) is then an
    # exp that is already done or in flight, so the per-slice dependency
    # cycle no longer contains a fresh exp and the loop runs ACT-paced.
    proj(0)
    proj(1)
    den0_next = 1
    for t in range(8):
        ready = [(blk, g) for blk in range(NBLK) for g in range(NGRP)
                 if (blk, g) not in pipe.emit_idx
                 and t_req[g] <= t and blk <= t + 1]
        for blk, g in sorted(ready)[:2]:
            emit_one(blk, g)
        if t + 2 < 8:
            proj(t + 2)
        if t >= 1:
            transposes(t - 1)
        # trickle block 0's deferred den chain behind its exps (max 2 per
        # step, each at least 2 emissions old, so the DVE never stalls
        # waiting for an in-flight exp ahead of the projection copies)
        for _ in range(2):
            if (den0_next < NGRP and (0, den0_next) in pipe.emit_idx
                    and pipe.emit_idx[(0, den0_next)] <= emitted - 2):
                pipe.den_add(0, den0_next)
                den0_next += 1
            else:
                break
    transposes(7)
    for g in range(NGRP):
        if (0, g) not in pipe.emit_idx:
            emit_one(0, g)
    # rest of block 0's deferred denominator chain
    for g in range(den0_next, NGRP):
        pipe.den_add(0, g)
    rest = [(blk, g) for blk in range(NBLK) for g in range(NGRP)
            if (blk, g) not in pipe.emit_idx]
    for blk, g in sorted(rest):
        emit_one(blk, g)
    pipe.drain_av(emitted, all_=True)


_CACHE = {}


def _build():
    if "nc" in _CACHE:
        return _CACHE["nc"]
    nc = bacc.Bacc("TRN2", target_bir_lowering=False, debug=False,
                   enable_asserts=False, num_devices=1)
    ins = {
        "xin": nc.dram_tensor("xin", [C, N], F32, kind="ExternalInput").ap(),
        "thw": nc.dram_tensor("thw", [C, IC], F32, kind="ExternalInput").ap(),
        "phw": nc.dram_tensor("phw", [C, IC], F32, kind="ExternalInput").ap(),
        "gw": nc.dram_tensor("gw", [C, IC], F32, kind="ExternalInput").ap(),
        "wef": nc.dram_tensor("wef", [IC, C], F32, kind="ExternalInput").ap(),
        "tb": nc.dram_tensor("tb", [IC], F32, kind="ExternalInput").ap(),
        "bfin": nc.dram_tensor("bfin", [C], F32, kind="ExternalInput").ap(),
        "ident": nc.dram_tensor("ident", [P, P], F32,
                                kind="ExternalInput").ap(),
    }
    yout = nc.dram_tensor("yout", [C, HALF], F32, kind="ExternalOutput").ap()
    with tile.TileContext(nc) as tc:
        with ExitStack() as ctx:
            _kernel_body(ctx, tc, ins, yout)
    nc.compile()
    _CACHE["nc"] = nc
    return nc


def _host_prepare(inputs):
    """Host-side folds + per-core input maps."""
    ii = {k: np.ascontiguousarray(np.asarray(v, dtype=np.float32))
          for k, v in inputs.items()}
    inv = ii["bn_gamma"] / np.sqrt(ii["bn_var"] + BN_EPS)
    w_eff = ii["w_w"] * inv[:, None]                       # [C, IC]
    b_final = (w_eff @ ii["g_b"] + ii["w_b"] * inv
               + ii["bn_beta"] - ii["bn_mean"] * inv)      # [C]
    shared = {
        "thw": np.ascontiguousarray(ii["theta_w"].T),      # [C, IC]
        "phw": np.ascontiguousarray(ii["phi_w"].T),
        "gw": np.ascontiguousarray(ii["g_w"].T),
        "wef": np.ascontiguousarray(w_eff.T),              # [IC, C]
        "tb": ii["theta_b"],
        "bfin": np.ascontiguousarray(b_final),
        "ident": np.eye(P, dtype=np.float32),
    }
    x = ii["x"].reshape(B, C, N)
    in_maps = []
    for core in range(NCORES):
        b, h = divmod(core, 2)
        own = x[b][:, h * HALF:(h + 1) * HALF]
        oth = x[b][:, (1 - h) * HALF:(2 - h) * HALF]
        xin = np.ascontiguousarray(np.concatenate([own, oth], axis=1))
        in_maps.append({"xin": xin, **shared})
    return in_maps


def _gather(results, x_dtype):
    out = np.empty((B, C, N), dtype=np.float32)
    for core in range(NCORES):
        b, h = divmod(core, 2)
        out[b][:, h * HALF:(h + 1) * HALF] = results[core]["yout"]
    return out.reshape(B, C, H, W).astype(x_dtype, copy=False)


def kernel(**inputs):
    nc = _build()
    in_maps = _host_prepare(inputs)
    res = run_bass_kernel_spmd(nc, in_maps, core_ids=list(range(NCORES)))
    return _gather(res.results, np.asarray(inputs["x"]).dtype)


# revision 34
# speedup vs baseline: 1.0162x; 1.0162x over previous
# NonLocalBlock Trainium2 Bass kernel.
#
# Reference computation (per batch b):
#   theta = theta_w @ X + theta_b          [IC, N]   (X = x[b] as [C, N])
#   phi   = phi_w   @ X + phi_b            [IC, N]
#   g     = g_w     @ X + g_b              [IC, N]
#   attn  = softmax_j(theta^T phi)         [N, N]
#   att   = g @ attn^T                     [IC, N]
#   y     = BN(w_w @ att + w_b) + x
#
# Math folds used on device (validated vs reference):
#   - phi bias drops out of softmax entirely (adds an i-only constant).
#   - g bias folds into the final bias because attn rows sum to 1.
#   - BN is affine: fold into w_eff = inv*w_w and b_final.
#   - scores bounded (|s| < 50) so exp() needs no max-subtraction.
#
# Sharding: 8 cores = 4 batches x 2 row-halves. Each core receives x[b]
# with its own half's columns swapped to the front, so every core runs an
# identical program (pure SPMD).
#
# v3 design (from HW traces of v1 @133us, v2 @119us):
#   - ACT exp() is the hard floor (~66us busy at [128,1536] groups); the
#     whole schedule exists to keep it saturated from ~10us on.
#   - All 44 (block x j-group) score groups are emitted inside the 8-slice
#     x-load loop, interleaved across blocks in (group, block) order: one
#     phi slice unlocks scores for every ready block at once, so the ACT
#     backlog absorbs any PE/DVE jitter (v2 lost ~25us in phase 1 to an
#     in-order DVE queue ping-ponging with ACT).
#   - Softmax denominator: exp emits bf16; per-block partial sums run as
#     TWO independent accumulator chains - groups 0-4 on the DVE, groups
#     5-10 on the otherwise-idle GPSIMD - merged on GPSIMD, then one
#     3x512-moving ones-matmul per block folds across partitions. This
#     keeps the DVE (~19us of unavoidable 1x-rate PSUM-source proj
#     copies) well under the ACT floor.
#   - Residual+bias fused into one scalar_tensor_tensor per store:
#     yo = (y_psum + b_final[k]) + x  (kills the xb precompute adds).
#   - PSUM: sc 2x[128,1536] (6 banks) + att 1 + tail 1. Phase-1
#     projections batch th|ph|g into one sc-sized slot per slice;
#     g-transposes batch 4 chunks into the tail bank.
#   - AV consumes drain block-major (att accumulates in a single PSUM
#     bank per block); on a block's last AV, its tail (den-fold, recip,
#     normalize, W, store) is emitted inline and hides under the
#     ACT-bound stream.

from collections import deque
from contextlib import ExitStack

import numpy as np

import concourse.bass as bass
import concourse.tile as tile
from concourse import bacc, mybir
from concourse.bass_utils import run_bass_kernel_spmd

F32 = mybir.dt.float32
F32R = mybir.dt.float32r
BF16 = mybir.dt.bfloat16
AF = mybir.ActivationFunctionType
ADD = mybir.AluOpType.add

B, C, IC = 4, 256, 128
H = W = 64
N = H * W            # 4096
HALF = N // 2        # 2048 rows of attention per core
P = 128
NCORES = 8
NBLK = HALF // 512   # 4 i-blocks of 512
NCH = N // P         # 32 j-chunks of 128
# groups of j-chunks per block: 10x3 + 1x2 = 32 chunks
GRPS = [list(range(3 * g, 3 * g + 3)) for g in range(10)] + [[30, 31]]
NGRP = len(GRPS)     # 11
DEFER = 2            # consume exp output this many groups late
NWARM = 6            # HAM warmup matmuls at t=0 (512-col)
EXBUFS = 20
BN_EPS = 1e-5


def _r(ap):
    return ap.bitcast(F32R)


class _Pipe:
    """Emission state for the flat (group, block)-interleaved pipeline."""

    def __init__(self, nc, pools, yout):
        self.nc = nc
        self.p = pools
        self.yout = yout
        self.av_q = [deque() for _ in range(NBLK)]
        self.av_blk = 0      # avs drain block-major
        self.gt_ready = 0    # j-chunks whose gT transpose has been emitted
        self.att = {}
        self.accd = {}
        self.exs = {}
        self.emit_idx = {}

    # ---- scores + exp + denominator chain links ------------------------
    def emit_group(self, blk, g):
        nc, p = self.nc, self.p
        chunks = GRPS[g]
        w = 512 * len(chunks)
        isl = slice(blk * 512, (blk + 1) * 512)
        sc = p["ps"].tile([P, 1536], F32, name=f"sc{blk}_{g}", tag="sc",
                          bufs=2)
        for pos, c in enumerate(chunks):
            nc.tensor.matmul(
                sc[:, pos * 512:(pos + 1) * 512],
                p["phi_sb"][:, c * P:(c + 1) * P],
                p["theta_sb"][:, isl],
                start=True, stop=True)
        ex = p["ex"].tile([P, 1536], BF16, name=f"ex{blk}_{g}", tag="ex",
                          bufs=EXBUFS)
        nc.scalar.activation(ex[:, :w], sc[:, :w], AF.Exp)
        self.exs[(blk, g)] = ex
        # denominator partial-sum chain (DVE, bf16 2x rate). Block 0's
        # chain is NOT emitted here: its groups emit during the x-load
        # loop where the DVE is busy with projection copies - the caller
        # bursts its chain right after the loop instead.
        if blk != 0:
            self.den_add(blk, g)
        self.av_q[blk].append((blk, g, ex))

    def den_add(self, blk, g):
        nc, p = self.nc, self.p
        ex = self.exs[(blk, g)]
        w = 512 * len(GRPS[g])
        if g == 0:
            return
        if g == 1:
            self.accd[blk] = p["rec"].tile(
                [P, 1536], BF16, name=f"acd{blk}", tag="accd", bufs=2)
            nc.vector.tensor_add(self.accd[blk][:],
                                 self.exs[(blk, 0)][:], ex[:])
        else:
            nc.vector.tensor_add(self.accd[blk][:, :w],
                                 self.accd[blk][:, :w], ex[:, :w])

    # ---- AV consume (g @ attn^T numerator) -----------------------------
    def _emit_av(self, blk, g, ex):
        nc, p = self.nc, self.p
        if g == 0:
            self.att[blk] = p["ps"].tile([P, 512], F32, name=f"att{blk}",
                                         tag="att", bufs=1)
        for pos, c in enumerate(GRPS[g]):
            nc.tensor.matmul(
                self.att[blk][:],
                p["gT_sb"][:, c * P:(c + 1) * P],
                ex[:, pos * 512:(pos + 1) * 512],
                start=c == 0, stop=c == NCH - 1)
        if g == NGRP - 1:
            self._emit_tail(blk)
            self.av_blk += 1

    def drain_av(self, emitted, all_=False):
        """Drain pending avs block-major; an av is eligible once its exp
        was emitted DEFER group-emissions ago AND its gT chunks' transposes
        have been emitted (emission order IS dependency order - a read
        emitted before its writer silently reads stale SBUF). At most 2
        per call so a backlog never opens a multi-us PE gap in the score
        stream."""
        n = 0
        while self.av_blk < NBLK and (all_ or n < 3):
            q = self.av_q[self.av_blk]
            if not q:
                break
            blk, g, ex = q[0]
            if GRPS[g][-1] >= self.gt_ready:
                break
            idx = self.emit_idx[(blk, g)]
            if not all_ and idx > emitted - DEFER:
                break
            q.popleft()
            self._emit_av(blk, g, ex)
            n += 1

    # ---- block tail: den-fold, recip, normalize, W, store --------------
    def _emit_tail(self, blk):
        nc, p = self.nc, self.p
        acc = self.accd[blk]
        den = p["ps"].tile([P, 512], F32, name=f"den{blk}", tag="tail",
                           bufs=1)
        for s in range(3):
            nc.tensor.matmul(den[:], p["onesb"][:],
                             acc[:, s * 512:(s + 1) * 512],
                             start=s == 0, stop=s == 2)
        recb = p["rec"].tile([P, 512], F32, name=f"recb{blk}", tag="recb",
                             bufs=2)
        nc.vector.reciprocal_approx_fast(out=recb[:], in_=den[:])
        attn = p["rec"].tile([P, 512], F32R, name=f"attn{blk}", tag="attn",
                             bufs=2)
        nc.vector.tensor_mul(attn[:], self.att[blk][:], recb[:])
        isl = slice(blk * 512, (blk + 1) * 512)
        for k in range(2):
            y = p["ps"].tile([P, 512], F32, name=f"y{blk}_{k}", tag="tail",
                             bufs=1)
            nc.tensor.matmul(y[:], p["wef_sb"][:, k * P:(k + 1) * P],
                             attn[:], start=True, stop=True)
            yo = p["rec"].tile([P, 512], F32, name=f"yo{blk}_{k}", tag="yo",
                               bufs=2)
            nc.vector.scalar_tensor_tensor(
                out=yo[:], in0=y[:], scalar=p["bfin_sb"][:, k:k + 1],
                in1=p["x_sb"][k][:, isl], op0=ADD, op1=ADD)
            nc.sync.dma_start(out=self.yout[k * P:(k + 1) * P, isl],
                              in_=yo[:])


def _kernel_body(ctx, tc, ins, yout):
    nc = tc.nc
    xin, thw, phw, gw, wef, tb, bfin = (
        ins["xin"], ins["thw"], ins["phw"], ins["gw"], ins["wef"],
        ins["tb"], ins["bfin"])

    consts = ctx.enter_context(tc.tile_pool(name="consts", bufs=1))
    big = ctx.enter_context(tc.tile_pool(name="big", bufs=1))

    # ---- dummies; exdum goes first on the ACT queue so walrus's
    # auto-inserted exp table load lands in the preamble window.
    dum_r = consts.tile([P, 512], F32R, name="dum_r")
    nc.vector.memset(dum_r.bitcast(F32)[:], 1.0)
    exdum = consts.tile([P, 1], F32, name="exdum")
    nc.scalar.activation(exdum[:], dum_r.bitcast(F32)[:, 0:1], AF.Exp)

    # ---- x load: 512-col slices, both HWDGE rings in parallel ----
    x_sb = [big.tile([P, N], F32R, name=f"x_sb{k}") for k in range(2)]

    # k=1 ring rides the GpSimd queue: the Scalar queue must stay
    # exp-only (a DMA descriptor issue costs ~650ns of ACT queue time
    # and serializes ahead of the exps).
    def xdma(t, k):
        tsl = slice(t * 512, (t + 1) * 512)
        eng = nc.sync if k == 0 else nc.gpsimd
        eng.dma_start(out=x_sb[k][:, tsl],
                      in_=_r(xin[k * P:(k + 1) * P, tsl]))

    for k in range(2):
        xdma(0, k)
    thw_sb = consts.tile([P, C], F32R, name="thw_sb")
    phw_sb = consts.tile([P, C], F32R, name="phw_sb")
    gw_sb = consts.tile([P, C], F32R, name="gw_sb")
    for k in range(2):
        nc.sync.dma_start(out=thw_sb[:, k * P:(k + 1) * P],
                          in_=_r(thw[k * P:(k + 1) * P, :]))
        nc.gpsimd.dma_start(out=phw_sb[:, k * P:(k + 1) * P],
                            in_=_r(phw[k * P:(k + 1) * P, :]))
        nc.sync.dma_start(out=gw_sb[:, k * P:(k + 1) * P],
                          in_=_r(gw[k * P:(k + 1) * P, :]))
    tb_sb = consts.tile([P, 1], F32, name="tb_sb")
    nc.gpsimd.dma_start(out=tb_sb[:], in_=tb[:, None])
    ident = consts.tile([P, P], F32, name="ident")
    nc.sync.dma_start(out=ident[:], in_=ins["ident"][:, :])
    for t in range(1, 8):
        for k in range(2):
            xdma(t, k)
    wef_sb = consts.tile([P, C], F32R, name="wef_sb")
    nc.sync.dma_start(out=wef_sb[:], in_=_r(wef[:, :]))
    bfin_sb = consts.tile([P, 2], F32, name="bfin_sb")
    nc.sync.dma_start(out=bfin_sb[:], in_=bfin.rearrange("(k p) -> p k", p=P))
    onesb = consts.tile([P, P], BF16, name="onesb")
    nc.vector.memset(onesb[:], 1.0)

    theta_sb = big.tile([P, HALF], F32R, name="theta_sb")
    phi_sb = big.tile([P, N], F32R, name="phi_sb")
    g_sb = big.tile([P, N], F32, name="g_sb")
    gT_sb = big.tile([P, N], BF16, name="gT_sb")

    # ---- PSUM: sc 2x[128,1536]=6 banks, att 1, tail 1 ----
    ps_pool = ctx.enter_context(tc.tile_pool(name="ps", bufs=1, space="PSUM"))
    pools = {
        "ps": ps_pool,
        "ex": ctx.enter_context(tc.tile_pool(name="ex", bufs=EXBUFS)),
        "rec": ctx.enter_context(tc.tile_pool(name="rec", bufs=2)),
        "theta_sb": theta_sb, "phi_sb": phi_sb, "gT_sb": gT_sb,
        "onesb": onesb, "wef_sb": wef_sb, "bfin_sb": bfin_sb, "x_sb": x_sb,
    }
    pipe = _Pipe(nc, pools, yout)

    # ---- HAM warmup while the first x slice streams in ----
    dum_ps = ps_pool.tile([P, 512], F32, name="dum_ps", tag="tail", bufs=1)
    for i in range(NWARM):
        nc.tensor.matmul(dum_ps[:], dum_r[:, 0:P], dum_r[:],
                         start=True, stop=True)

    # ---- per-slice projections batched into one sc-sized PSUM slot
    # (th | ph | g); g-transposes batch 4 chunks into the tail bank.
    def proj(t):
        tsl = slice(t * 512, (t + 1) * 512)
        slot = ps_pool.tile([P, 1536], F32, name=f"proj{t}", tag="sc",
                            bufs=2)
        if t < NBLK:
            for k in range(2):
                nc.tensor.matmul(slot[:, 0:512],
                                 thw_sb[:, k * P:(k + 1) * P],
                                 x_sb[k][:, tsl],
                                 start=k == 0, stop=k == 1)
        for k in range(2):
            nc.tensor.matmul(slot[:, 512:1024],
                             phw_sb[:, k * P:(k + 1) * P],
                             x_sb[k][:, tsl],
                             start=k == 0, stop=k == 1)
        for k in range(2):
            nc.tensor.matmul(slot[:, 1024:1536],
                             gw_sb[:, k * P:(k + 1) * P],
                             x_sb[k][:, tsl],
                             start=k == 0, stop=k == 1)
        # phi first: it is the copy that gates the next score groups
        nc.vector.tensor_copy(phi_sb[:, tsl], slot[:, 512:1024])
        if t < NBLK:
            nc.vector.tensor_scalar_add(theta_sb[:, tsl], slot[:, 0:512],
                                        tb_sb[:])
        nc.vector.tensor_copy(g_sb[:, tsl], slot[:, 1024:1536])

    def transposes(t):
        pst = ps_pool.tile([P, 512], F32, name=f"gt{t}", tag="tail", bufs=1)
        for j in range(4):
            jc = 4 * t + j
            nc.tensor.transpose(pst[:, j * P:(j + 1) * P],
                                g_sb[:, jc * P:(jc + 1) * P], ident[:])
        nc.vector.tensor_copy(gT_sb[:, 4 * t * P:(4 * t + 4) * P], pst[:])
        pipe.gt_ready = 4 * (t + 1)

    # group g needs phi slice (last_chunk // 4); block blk needs theta
    # slice blk. In the slice loop emit at most TWO ready groups per slice
    # (block-major priority) - the proj slot then rotates onto a slot
    # whose last reader is cheap, so projections track the DMA stream
    # instead of slipping to exp pace. The remaining groups bulk-emit
    # after the loop, block-major, paced by the sc slots.
    t_req = [GRPS[g][-1] // 4 for g in range(NGRP)]
    emitted = 0

    def emit_one(blk, g):
        nonlocal emitted
        # drain BEFORE the scores: pending av work then fills the PE's
        # wait for the sc slot's previous exp instead of queuing behind it
        pipe.drain_av(emitted)
        pipe.emit_idx[(blk, g)] = emitted
        pipe.emit_group(blk, g)
        emitted += 1

    # Projections run ONE SLICE AHEAD of their score groups: a proj's sc
    # slot predecessor (2 allocations back in the rotation) is then an
    # exp that is already done or in flight, so the per-slice dependency
    # cycle no longer contains a fresh exp and the loop runs ACT-paced.
    proj(0)
    proj(1)
    den0_next = 1
    for t in range(8):
        ready = [(blk, g) for blk in range(NBLK) for g in range(NGRP)
                 if (blk, g) not in pipe.emit_idx
                 and t_req[g] <= t and blk <= t + 1]
        for blk, g in sorted(ready)[:2]:
            emit_one(blk, g)
        if t + 2 < 8:
            proj(t + 2)
        if t >= 1:
            transposes(t - 1)
        # trickle block 0's deferred den chain behind its exps (max 2 per
        # step, each at least 2 emissions old, so the DVE never stalls
        # waiting for an in-flight exp ahead of the projection copies)
        for _ in range(2):
            if (den0_next < NGRP and (0, den0_next) in pipe.emit_idx
                    and pipe.emit_idx[(0, den0_next)] <= emitted - 2):
                pipe.den_add(0, den0_next)
                den0_next += 1
            else:
                break
    transposes(7)
    for g in range(NGRP):
        if (0, g) not in pipe.emit_idx:
            emit_one(0, g)
    # rest of block 0's deferred denominator chain
    for g in range(den0_next, NGRP):
        pipe.den_add(0, g)
    rest = [(blk, g) for blk in range(NBLK) for g in range(NGRP)
            if (blk, g) not in pipe.emit_idx]
    for blk, g in sorted(rest):
        emit_one(blk, g)
    pipe.drain_av(emitted, all_=True)


_CACHE = {}


def _build():
    if "nc" in _CACHE:
        return _CACHE["nc"]
    nc = bacc.Bacc("TRN2", target_bir_lowering=False, debug=False,
                   enable_asserts=False, num_devices=1)
    ins = {
        "xin": nc.dram_tensor("xin", [C, N], F32, kind="ExternalInput").ap(),
        "thw": nc.dram_tensor("thw", [C, IC], F32, kind="ExternalInput").ap(),
        "phw": nc.dram_tensor("phw", [C, IC], F32, kind="ExternalInput").ap(),
        "gw": nc.dram_tensor("gw", [C, IC], F32, kind="ExternalInput").ap(),
        "wef": nc.dram_tensor("wef", [IC, C], F32, kind="ExternalInput").ap(),
        "tb": nc.dram_tensor("tb", [IC], F32, kind="ExternalInput").ap(),
        "bfin": nc.dram_tensor("bfin", [C], F32, kind="ExternalInput").ap(),
        "ident": nc.dram_tensor("ident", [P, P], F32,
                                kind="ExternalInput").ap(),
    }
    yout = nc.dram_tensor("yout", [C, HALF], F32, kind="ExternalOutput").ap()
    with tile.TileContext(nc) as tc:
        with ExitStack() as ctx:
            _kernel_body(ctx, tc, ins, yout)
    nc.compile()
    _CACHE["nc"] = nc
    return nc


def _host_prepare(inputs):
    """Host-side folds + per-core input maps."""
    ii = {k: np.ascontiguousarray(np.asarray(v, dtype=np.float32))
          for k, v in inputs.items()}
    inv = ii["bn_gamma"] / np.sqrt(ii["bn_var"] + BN_EPS)
    w_eff = ii["w_w"] * inv[:, None]                       # [C, IC]
    b_final = (w_eff @ ii["g_b"] + ii["w_b"] * inv
               + ii["bn_beta"] - ii["bn_mean"] * inv)      # [C]
    shared = {
        "thw": np.ascontiguousarray(ii["theta_w"].T),      # [C, IC]
        "phw": np.ascontiguousarray(ii["phi_w"].T),
        "gw": np.ascontiguousarray(ii["g_w"].T),
        "wef": np.ascontiguousarray(w_eff.T),              # [IC, C]
        "tb": ii["theta_b"],
        "bfin": np.ascontiguousarray(b_final),
        "ident": np.eye(P, dtype=np.float32),
    }
    x = ii["x"].reshape(B, C, N)
    in_maps = []
    for core in range(NCORES):
        b, h = divmod(core, 2)
        own = x[b][:, h * HALF:(h + 1) * HALF]
        oth = x[b][:, (1 - h) * HALF:(2 - h) * HALF]
        xin = np.ascontiguousarray(np.concatenate([own, oth], axis=1))
        in_maps.append({"xin": xin, **shared})
    return in_maps


def _gather(results, x_dtype):
    out = np.empty((B, C, N), dtype=np.float32)
    for core in range(NCORES):
        b, h = divmod(core, 2)
        out[b][:, h * HALF:(h + 1) * HALF] = results[core]["yout"]
    return out.reshape(B, C, H, W).astype(x_dtype, copy=False)


def kernel(**inputs):
    nc = _build()
    in_maps = _host_prepare(inputs)
    res = run_bass_kernel_spmd(nc, in_maps, core_ids=list(range(NCORES)))
    return _gather(res.results, np.asarray(inputs["x"]).dtype)


# revision 37
# speedup vs baseline: 1.0237x; 1.0074x over previous
# NonLocalBlock Trainium2 Bass kernel.
#
# Reference computation (per batch b):
#   theta = theta_w @ X + theta_b          [IC, N]   (X = x[b] as [C, N])
#   phi   = phi_w   @ X + phi_b            [IC, N]
#   g     = g_w     @ X + g_b              [IC, N]
#   attn  = softmax_j(theta^T phi)         [N, N]
#   att   = g @ attn^T                     [IC, N]
#   y     = BN(w_w @ att + w_b) + x
#
# Math folds used on device (validated vs reference):
#   - phi bias drops out of softmax entirely (adds an i-only constant).
#   - g bias folds into the final bias because attn rows sum to 1.
#   - BN is affine: fold into w_eff = inv*w_w and b_final.
#   - scores bounded (|s| < 50) so exp() needs no max-subtraction.
#
# Sharding: 8 cores = 4 batches x 2 row-halves. Each core receives x[b]
# with its own half's columns swapped to the front, so every core runs an
# identical program (pure SPMD).
#
# v3 design (from HW traces of v1 @133us, v2 @119us):
#   - ACT exp() is the hard floor (~66us busy at [128,1536] groups); the
#     whole schedule exists to keep it saturated from ~10us on.
#   - All 44 (block x j-group) score groups are emitted inside the 8-slice
#     x-load loop, interleaved across blocks in (group, block) order: one
#     phi slice unlocks scores for every ready block at once, so the ACT
#     backlog absorbs any PE/DVE jitter (v2 lost ~25us in phase 1 to an
#     in-order DVE queue ping-ponging with ACT).
#   - Softmax denominator: exp emits bf16; per-block partial sums run as
#     TWO independent accumulator chains - groups 0-4 on the DVE, groups
#     5-10 on the otherwise-idle GPSIMD - merged on GPSIMD, then one
#     3x512-moving ones-matmul per block folds across partitions. This
#     keeps the DVE (~19us of unavoidable 1x-rate PSUM-source proj
#     copies) well under the ACT floor.
#   - Residual+bias fused into one scalar_tensor_tensor per store:
#     yo = (y_psum + b_final[k]) + x  (kills the xb precompute adds).
#   - PSUM: sc 2x[128,1536] (6 banks) + att 1 + tail 1. Phase-1
#     projections batch th|ph|g into one sc-sized slot per slice;
#     g-transposes batch 4 chunks into the tail bank.
#   - AV consumes drain block-major (att accumulates in a single PSUM
#     bank per block); on a block's last AV, its tail (den-fold, recip,
#     normalize, W, store) is emitted inline and hides under the
#     ACT-bound stream.

from collections import deque
from contextlib import ExitStack

import numpy as np

import concourse.bass as bass
import concourse.tile as tile
from concourse import bacc, mybir
from concourse.bass_utils import run_bass_kernel_spmd

F32 = mybir.dt.float32
F32R = mybir.dt.float32r
BF16 = mybir.dt.bfloat16
AF = mybir.ActivationFunctionType
ADD = mybir.AluOpType.add

B, C, IC = 4, 256, 128
H = W = 64
N = H * W            # 4096
HALF = N // 2        # 2048 rows of attention per core
P = 128
NCORES = 8
NBLK = HALF // 512   # 4 i-blocks of 512
NCH = N // P         # 32 j-chunks of 128
# groups of j-chunks per block: 10x3 + 1x2 = 32 chunks
GRPS = [list(range(3 * g, 3 * g + 3)) for g in range(10)] + [[30, 31]]
NGRP = len(GRPS)     # 11
DEFER = 2            # consume exp output this many groups late
NWARM = 6            # HAM warmup matmuls at t=0 (512-col)
EXBUFS = 20
BN_EPS = 1e-5


def _r(ap):
    return ap.bitcast(F32R)


class _Pipe:
    """Emission state for the flat (group, block)-interleaved pipeline."""

    def __init__(self, nc, pools, yout):
        self.nc = nc
        self.p = pools
        self.yout = yout
        self.av_q = [deque() for _ in range(NBLK)]
        self.av_blk = 0      # avs drain block-major
        self.gt_ready = 0    # j-chunks whose gT transpose has been emitted
        self.att = {}
        self.accd = {}
        self.exs = {}
        self.emit_idx = {}

    # ---- scores + exp + denominator chain links ------------------------
    def emit_group(self, blk, g):
        nc, p = self.nc, self.p
        chunks = GRPS[g]
        w = 512 * len(chunks)
        isl = slice(blk * 512, (blk + 1) * 512)
        sc = p["ps"].tile([P, 1536], F32, name=f"sc{blk}_{g}", tag="sc",
                          bufs=2)
        for pos, c in enumerate(chunks):
            nc.tensor.matmul(
                sc[:, pos * 512:(pos + 1) * 512],
                p["phi_sb"][:, c * P:(c + 1) * P],
                p["theta_sb"][:, isl],
                start=True, stop=True)
        ex = p["ex"].tile([P, 1536], BF16, name=f"ex{blk}_{g}", tag="ex",
                          bufs=EXBUFS)
        nc.scalar.activation(ex[:, :w], sc[:, :w], AF.Exp)
        self.exs[(blk, g)] = ex
        # denominator partial-sum chain (DVE, bf16 2x rate). Block 0's
        # chain is NOT emitted here: its groups emit during the x-load
        # loop where the DVE is busy with projection copies - the caller
        # bursts its chain right after the loop instead.
        if blk != 0:
            self.den_add(blk, g)
        self.av_q[blk].append((blk, g, ex))

    def den_add(self, blk, g):
        nc, p = self.nc, self.p
        ex = self.exs[(blk, g)]
        w = 512 * len(GRPS[g])
        if g == 0:
            return
        if g == 1:
            self.accd[blk] = p["rec"].tile(
                [P, 1536], BF16, name=f"acd{blk}", tag="accd", bufs=2)
            nc.vector.tensor_add(self.accd[blk][:],
                                 self.exs[(blk, 0)][:], ex[:])
        else:
            nc.vector.tensor_add(self.accd[blk][:, :w],
                                 self.accd[blk][:, :w], ex[:, :w])

    # ---- AV consume (g @ attn^T numerator) -----------------------------
    def _emit_av(self, blk, g, ex):
        nc, p = self.nc, self.p
        if g == 0:
            self.att[blk] = p["ps"].tile([P, 512], F32, name=f"att{blk}",
                                         tag="att", bufs=1)
        for pos, c in enumerate(GRPS[g]):
            nc.tensor.matmul(
                self.att[blk][:],
                p["gT_sb"][:, c * P:(c + 1) * P],
                ex[:, pos * 512:(pos + 1) * 512],
                start=c == 0, stop=c == NCH - 1)
        if g == NGRP - 1:
            self._emit_tail(blk)
            self.av_blk += 1

    def drain_av(self, emitted, all_=False):
        """Drain pending avs block-major; an av is eligible once its exp
        was emitted DEFER group-emissions ago AND its gT chunks' transposes
        have been emitted (emission order IS dependency order - a read
        emitted before its writer silently reads stale SBUF). At most 2
        per call so a backlog never opens a multi-us PE gap in the score
        stream."""
        n = 0
        while self.av_blk < NBLK and (all_ or n < 2):
            q = self.av_q[self.av_blk]
            if not q:
                break
            blk, g, ex = q[0]
            if GRPS[g][-1] >= self.gt_ready:
                break
            idx = self.emit_idx[(blk, g)]
            if not all_ and idx > emitted - DEFER:
                break
            q.popleft()
            self._emit_av(blk, g, ex)
            n += 1

    # ---- block tail: den-fold, recip, normalize, W, store --------------
    def _emit_tail(self, blk):
        nc, p = self.nc, self.p
        acc = self.accd[blk]
        den = p["ps"].tile([P, 512], F32, name=f"den{blk}", tag="tail",
                           bufs=1)
        for s in range(3):
            nc.tensor.matmul(den[:], p["onesb"][:],
                             acc[:, s * 512:(s + 1) * 512],
                             start=s == 0, stop=s == 2)
        recb = p["rec"].tile([P, 512], F32, name=f"recb{blk}", tag="recb",
                             bufs=2)
        nc.vector.reciprocal_approx_fast(out=recb[:], in_=den[:])
        attn = p["rec"].tile([P, 512], F32R, name=f"attn{blk}", tag="attn",
                             bufs=2)
        nc.vector.tensor_mul(attn[:], self.att[blk][:], recb[:])
        isl = slice(blk * 512, (blk + 1) * 512)
        for k in range(2):
            y = p["ps"].tile([P, 512], F32, name=f"y{blk}_{k}", tag="tail",
                             bufs=1)
            nc.tensor.matmul(y[:], p["wef_sb"][:, k * P:(k + 1) * P],
                             attn[:], start=True, stop=True)
            yo = p["rec"].tile([P, 512], F32, name=f"yo{blk}_{k}", tag="yo",
                               bufs=2)
            nc.vector.scalar_tensor_tensor(
                out=yo[:], in0=y[:], scalar=p["bfin_sb"][:, k:k + 1],
                in1=p["x_sb"][k][:, isl], op0=ADD, op1=ADD)
            nc.sync.dma_start(out=self.yout[k * P:(k + 1) * P, isl],
                              in_=yo[:])


def _kernel_body(ctx, tc, ins, yout):
    nc = tc.nc
    xin, thw, phw, gw, wef, tb, bfin = (
        ins["xin"], ins["thw"], ins["phw"], ins["gw"], ins["wef"],
        ins["tb"], ins["bfin"])

    consts = ctx.enter_context(tc.tile_pool(name="consts", bufs=1))
    big = ctx.enter_context(tc.tile_pool(name="big", bufs=1))

    # ---- dummies; exdum goes first on the ACT queue so walrus's
    # auto-inserted exp table load lands in the preamble window.
    dum_r = consts.tile([P, 512], F32R, name="dum_r")
    nc.vector.memset(dum_r.bitcast(F32)[:], 1.0)
    exdum = consts.tile([P, 1], F32, name="exdum")
    nc.scalar.activation(exdum[:], dum_r.bitcast(F32)[:, 0:1], AF.Exp)

    # ---- x load: 512-col slices, both HWDGE rings in parallel ----
    x_sb = [big.tile([P, N], F32R, name=f"x_sb{k}") for k in range(2)]

    # k=1 ring rides the GpSimd queue: the Scalar queue must stay
    # exp-only (a DMA descriptor issue costs ~650ns of ACT queue time
    # and serializes ahead of the exps).
    def xdma(t, k):
        tsl = slice(t * 512, (t + 1) * 512)
        eng = nc.sync if k == 0 else nc.gpsimd
        eng.dma_start(out=x_sb[k][:, tsl],
                      in_=_r(xin[k * P:(k + 1) * P, tsl]))

    for k in range(2):
        xdma(0, k)
    thw_sb = consts.tile([P, C], F32R, name="thw_sb")
    phw_sb = consts.tile([P, C], F32R, name="phw_sb")
    gw_sb = consts.tile([P, C], F32R, name="gw_sb")
    for k in range(2):
        nc.sync.dma_start(out=thw_sb[:, k * P:(k + 1) * P],
                          in_=_r(thw[k * P:(k + 1) * P, :]))
        nc.gpsimd.dma_start(out=phw_sb[:, k * P:(k + 1) * P],
                            in_=_r(phw[k * P:(k + 1) * P, :]))
        nc.sync.dma_start(out=gw_sb[:, k * P:(k + 1) * P],
                          in_=_r(gw[k * P:(k + 1) * P, :]))
    tb_sb = consts.tile([P, 1], F32, name="tb_sb")
    nc.gpsimd.dma_start(out=tb_sb[:], in_=tb[:, None])
    ident = consts.tile([P, P], F32, name="ident")
    nc.sync.dma_start(out=ident[:], in_=ins["ident"][:, :])
    for t in range(1, 8):
        for k in range(2):
            xdma(t, k)
    wef_sb = consts.tile([P, C], F32R, name="wef_sb")
    nc.sync.dma_start(out=wef_sb[:], in_=_r(wef[:, :]))
    bfin_sb = consts.tile([P, 2], F32, name="bfin_sb")
    nc.sync.dma_start(out=bfin_sb[:], in_=bfin.rearrange("(k p) -> p k", p=P))
    onesb = consts.tile([P, P], BF16, name="onesb")
    nc.vector.memset(onesb[:], 1.0)

    theta_sb = big.tile([P, HALF], F32R, name="theta_sb")
    phi_sb = big.tile([P, N], F32R, name="phi_sb")
    g_sb = big.tile([P, N], F32, name="g_sb")
    gT_sb = big.tile([P, N], BF16, name="gT_sb")

    # ---- PSUM: sc 2x[128,1536]=6 banks, att 1, tail 1 ----
    ps_pool = ctx.enter_context(tc.tile_pool(name="ps", bufs=1, space="PSUM"))
    pools = {
        "ps": ps_pool,
        "ex": ctx.enter_context(tc.tile_pool(name="ex", bufs=EXBUFS)),
        "rec": ctx.enter_context(tc.tile_pool(name="rec", bufs=2)),
        "theta_sb": theta_sb, "phi_sb": phi_sb, "gT_sb": gT_sb,
        "onesb": onesb, "wef_sb": wef_sb, "bfin_sb": bfin_sb, "x_sb": x_sb,
    }
    pipe = _Pipe(nc, pools, yout)

    # ---- HAM warmup while the first x slice streams in ----
    dum_ps = ps_pool.tile([P, 512], F32, name="dum_ps", tag="tail", bufs=1)
    for i in range(NWARM):
        nc.tensor.matmul(dum_ps[:], dum_r[:, 0:P], dum_r[:],
                         start=True, stop=True)

    # ---- per-slice projections batched into one sc-sized PSUM slot
    # (th | ph | g); g-transposes batch 4 chunks into the tail bank.
    def proj(t):
        tsl = slice(t * 512, (t + 1) * 512)
        slot = ps_pool.tile([P, 1536], F32, name=f"proj{t}", tag="sc",
                            bufs=2)
        if t < NBLK:
            for k in range(2):
                nc.tensor.matmul(slot[:, 0:512],
                                 thw_sb[:, k * P:(k + 1) * P],
                                 x_sb[k][:, tsl],
                                 start=k == 0, stop=k == 1)
        for k in range(2):
            nc.tensor.matmul(slot[:, 512:1024],
                             phw_sb[:, k * P:(k + 1) * P],
                             x_sb[k][:, tsl],
                             start=k == 0, stop=k == 1)
        for k in range(2):
            nc.tensor.matmul(slot[:, 1024:1536],
                             gw_sb[:, k * P:(k + 1) * P],
                             x_sb[k][:, tsl],
                             start=k == 0, stop=k == 1)
        # phi first: it is the copy that gates the next score groups
        nc.vector.tensor_copy(phi_sb[:, tsl], slot[:, 512:1024])
        if t < NBLK:
            nc.vector.tensor_scalar_add(theta_sb[:, tsl], slot[:, 0:512],
                                        tb_sb[:])
        nc.vector.tensor_copy(g_sb[:, tsl], slot[:, 1024:1536])

    def transposes(t):
        pst = ps_pool.tile([P, 512], F32, name=f"gt{t}", tag="tail", bufs=1)
        for j in range(4):
            jc = 4 * t + j
            nc.tensor.transpose(pst[:, j * P:(j + 1) * P],
                                g_sb[:, jc * P:(jc + 1) * P], ident[:])
        nc.vector.tensor_copy(gT_sb[:, 4 * t * P:(4 * t + 4) * P], pst[:])
        pipe.gt_ready = 4 * (t + 1)

    # group g needs phi slice (last_chunk // 4); block blk needs theta
    # slice blk. In the slice loop emit at most TWO ready groups per slice
    # (block-major priority) - the proj slot then rotates onto a slot
    # whose last reader is cheap, so projections track the DMA stream
    # instead of slipping to exp pace. The remaining groups bulk-emit
    # after the loop, block-major, paced by the sc slots.
    t_req = [GRPS[g][-1] // 4 for g in range(NGRP)]
    emitted = 0

    def emit_one(blk, g):
        nonlocal emitted
        pipe.emit_idx[(blk, g)] = emitted
        pipe.emit_group(blk, g)
        emitted += 1
        pipe.drain_av(emitted)

    # Projections run ONE SLICE AHEAD of their score groups: a proj's sc
    # slot predecessor (2 allocations back in the rotation) is then an
    # exp that is already done or in flight, so the per-slice dependency
    # cycle no longer contains a fresh exp and the loop runs ACT-paced.
    proj(0)
    proj(1)
    den0_next = 1
    for t in range(8):
        ready = [(blk, g) for blk in range(NBLK) for g in range(NGRP)
                 if (blk, g) not in pipe.emit_idx
                 and t_req[g] <= t and blk <= t + 1]
        for blk, g in sorted(ready)[:2]:
            emit_one(blk, g)
        if t + 2 < 8:
            proj(t + 2)
        if t >= 1:
            transposes(t - 1)
        if t >= 5:
            # trickle block 0's deferred den chain into late-loop DVE slack
            pipe.den_add(0, t - 4)
            den0_next = t - 3
    transposes(7)
    for g in range(NGRP):
        if (0, g) not in pipe.emit_idx:
            emit_one(0, g)
    # rest of block 0's deferred denominator chain
    for g in range(den0_next, NGRP):
        pipe.den_add(0, g)
    rest = [(blk, g) for blk in range(NBLK) for g in range(NGRP)
            if (blk, g) not in pipe.emit_idx]
    for blk, g in sorted(rest):
        emit_one(blk, g)
    pipe.drain_av(emitted, all_=True)


_CACHE = {}


def _build():
    if "nc" in _CACHE:
        return _CACHE["nc"]
    nc = bacc.Bacc("TRN2", target_bir_lowering=False, debug=False,
                   enable_asserts=False, num_devices=1)
    ins = {
        "xin": nc.dram_tensor("xin", [C, N], F32, kind="ExternalInput").ap(),
        "thw": nc.dram_tensor("thw", [C, IC], F32, kind="ExternalInput").ap(),
        "phw": nc.dram_tensor("phw", [C, IC], F32, kind="ExternalInput").ap(),
        "gw": nc.dram_tensor("gw", [C, IC], F32, kind="ExternalInput").ap(),
        "wef": nc.dram_tensor("wef", [IC, C], F32, kind="ExternalInput").ap(),
        "tb": nc.dram_tensor("tb", [IC], F32, kind="ExternalInput").ap(),
        "bfin": nc.dram_tensor("bfin", [C], F32, kind="ExternalInput").ap(),
        "ident": nc.dram_tensor("ident", [P, P], F32,
                                kind="ExternalInput").ap(),
    }
    yout = nc.dram_tensor("yout", [C, HALF], F32, kind="ExternalOutput").ap()
    with tile.TileContext(nc) as tc:
        with ExitStack() as ctx:
            _kernel_body(ctx, tc, ins, yout)
    nc.compile()
    _CACHE["nc"] = nc
    return nc


def _host_prepare(inputs):
    """Host-side folds + per-core input maps."""
    ii = {k: np.ascontiguousarray(np.asarray(v, dtype=np.float32))
          for k, v in inputs.items()}
    inv = ii["bn_gamma"] / np.sqrt(ii["bn_var"] + BN_EPS)
    w_eff = ii["w_w"] * inv[:, None]                       # [C, IC]
    b_final = (w_eff @ ii["g_b"] + ii["w_b"] * inv
               + ii["bn_beta"] - ii["bn_mean"] * inv)      # [C]
    shared = {
        "thw": np.ascontiguousarray(ii["theta_w"].T),      # [C, IC]
        "phw": np.ascontiguousarray(ii["phi_w"].T),
        "gw": np.ascontiguousarray(ii["g_w"].T),
        "wef": np.ascontiguousarray(w_eff.T),              # [IC, C]
        "tb": ii["theta_b"],
        "bfin": np.ascontiguousarray(b_final),
        "ident": np.eye(P, dtype=np.float32),
    }
    x = ii["x"].reshape(B, C, N)
    in_maps = []
    for core in range(NCORES):
        b, h = divmod(core, 2)
        own = x[b][:, h * HALF:(h + 1) * HALF]
        oth = x[b][:, (1 - h) * HALF:(2 - h) * HALF]
        xin = np.ascontiguousarray(np.concatenate([own, oth], axis=1))
        in_maps.append({"xin": xin, **shared})
    return in_maps


def _gather(results, x_dtype):
    out = np.empty((B, C, N), dtype=np.float32)
    for core in range(NCORES):
        b, h = divmod(core, 2)
        out[b][:, h * HALF:(h + 1) * HALF] = results[core]["yout"]
    return out.reshape(B, C, H, W).astype(x_dtype, copy=False)


def kernel(**inputs):
    nc = _build()
    in_maps = _host_prepare(inputs)
    res = run_bass_kernel_spmd(nc, in_maps, core_ids=list(range(NCORES)))
    return _gather(res.results, np.asarray(inputs["x"]).dtype)


# revision 41
# speedup vs baseline: 1.0361x; 1.0121x over previous
# NonLocalBlock Trainium2 Bass kernel.
#
# Reference computation (per batch b):
#   theta = theta_w @ X + theta_b          [IC, N]   (X = x[b] as [C, N])
#   phi   = phi_w   @ X + phi_b            [IC, N]
#   g     = g_w     @ X + g_b              [IC, N]
#   attn  = softmax_j(theta^T phi)         [N, N]
#   att   = g @ attn^T                     [IC, N]
#   y     = BN(w_w @ att + w_b) + x
#
# Math folds used on device (validated vs reference):
#   - phi bias drops out of softmax entirely (adds an i-only constant).
#   - g bias folds into the final bias because attn rows sum to 1.
#   - BN is affine: fold into w_eff = inv*w_w and b_final.
#   - scores bounded (|s| < 50) so exp() needs no max-subtraction.
#
# Sharding: 8 cores = 4 batches x 2 row-halves. Each core receives x[b]
# with its own half's columns swapped to the front, so every core runs an
# identical program (pure SPMD).
#
# v3 design (from HW traces of v1 @133us, v2 @119us):
#   - ACT exp() is the hard floor (~66us busy at [128,1536] groups); the
#     whole schedule exists to keep it saturated from ~10us on.
#   - All 44 (block x j-group) score groups are emitted inside the 8-slice
#     x-load loop, interleaved across blocks in (group, block) order: one
#     phi slice unlocks scores for every ready block at once, so the ACT
#     backlog absorbs any PE/DVE jitter (v2 lost ~25us in phase 1 to an
#     in-order DVE queue ping-ponging with ACT).
#   - Softmax denominator: exp emits bf16; per-block partial sums run as
#     TWO independent accumulator chains - groups 0-4 on the DVE, groups
#     5-10 on the otherwise-idle GPSIMD - merged on GPSIMD, then one
#     3x512-moving ones-matmul per block folds across partitions. This
#     keeps the DVE (~19us of unavoidable 1x-rate PSUM-source proj
#     copies) well under the ACT floor.
#   - Residual+bias fused into one scalar_tensor_tensor per store:
#     yo = (y_psum + b_final[k]) + x  (kills the xb precompute adds).
#   - PSUM: sc 2x[128,1536] (6 banks) + att 1 + tail 1. Phase-1
#     projections batch th|ph|g into one sc-sized slot per slice;
#     g-transposes batch 4 chunks into the tail bank.
#   - AV consumes drain block-major (att accumulates in a single PSUM
#     bank per block); on a block's last AV, its tail (den-fold, recip,
#     normalize, W, store) is emitted inline and hides under the
#     ACT-bound stream.

from collections import deque
from contextlib import ExitStack

import numpy as np

import concourse.bass as bass
import concourse.tile as tile
from concourse import bacc, mybir
from concourse.bass_utils import run_bass_kernel_spmd

F32 = mybir.dt.float32
F32R = mybir.dt.float32r
BF16 = mybir.dt.bfloat16
AF = mybir.ActivationFunctionType
ADD = mybir.AluOpType.add

B, C, IC = 4, 256, 128
H = W = 64
N = H * W            # 4096
HALF = N // 2        # 2048 rows of attention per core
P = 128
NCORES = 8
NBLK = HALF // 512   # 4 i-blocks of 512
NCH = N // P         # 32 j-chunks of 128
# groups of j-chunks per block: 10x3 + 1x2 = 32 chunks
GRPS = [list(range(3 * g, 3 * g + 3)) for g in range(10)] + [[30, 31]]
NGRP = len(GRPS)     # 11
DEFER = 2            # consume exp output this many groups late
NWARM = 6            # HAM warmup matmuls at t=0 (512-col)
EXBUFS = 20
BN_EPS = 1e-5


def _r(ap):
    return ap.bitcast(F32R)


class _Pipe:
    """Emission state for the flat (group, block)-interleaved pipeline."""

    def __init__(self, nc, pools, yout):
        self.nc = nc
        self.p = pools
        self.yout = yout
        self.av_q = [deque() for _ in range(NBLK)]
        self.av_blk = 0      # avs drain block-major
        self.gt_ready = 0    # j-chunks whose gT transpose has been emitted
        self.att = {}
        self.accd = {}
        self.exs = {}
        self.emit_idx = {}

    # ---- scores + exp + denominator chain links ------------------------
    def emit_group(self, blk, g):
        nc, p = self.nc, self.p
        chunks = GRPS[g]
        w = 512 * len(chunks)
        isl = slice(blk * 512, (blk + 1) * 512)
        sc = p["ps"].tile([P, 1536], F32, name=f"sc{blk}_{g}", tag="sc",
                          bufs=2)
        for pos, c in enumerate(chunks):
            nc.tensor.matmul(
                sc[:, pos * 512:(pos + 1) * 512],
                p["phi_sb"][:, c * P:(c + 1) * P],
                p["theta_sb"][:, isl],
                start=True, stop=True)
        ex = p["ex"].tile([P, 1536], BF16, name=f"ex{blk}_{g}", tag="ex",
                          bufs=EXBUFS)
        nc.scalar.activation(ex[:, :w], sc[:, :w], AF.Exp)
        self.exs[(blk, g)] = ex
        # denominator partial-sum chain (DVE, bf16 2x rate). Block 0's
        # chain is NOT emitted here: its groups emit during the x-load
        # loop where the DVE is busy with projection copies - the caller
        # bursts its chain right after the loop instead.
        if blk != 0:
            self.den_add(blk, g)
        self.av_q[blk].append((blk, g, ex))

    def den_add(self, blk, g):
        nc, p = self.nc, self.p
        ex = self.exs[(blk, g)]
        w = 512 * len(GRPS[g])
        if g == 0:
            return
        if blk == NBLK - 1 and g >= NGRP - 2:
            # last block: groups 9/10 skip the DVE chain; the den-ones
            # fold consumes their ex tiles directly, shortening the
            # end-of-kernel critical chain by a DVE add.
            return
        if g == 1:
            self.accd[blk] = p["rec"].tile(
                [P, 1536], BF16, name=f"acd{blk}", tag="accd", bufs=2)
            nc.vector.tensor_add(self.accd[blk][:],
                                 self.exs[(blk, 0)][:], ex[:])
        else:
            nc.vector.tensor_add(self.accd[blk][:, :w],
                                 self.accd[blk][:, :w], ex[:, :w])

    # ---- AV consume (g @ attn^T numerator) -----------------------------
    def _emit_av(self, blk, g, ex):
        nc, p = self.nc, self.p
        if g == 0:
            self.att[blk] = p["ps"].tile([P, 512], F32, name=f"att{blk}",
                                         tag="att", bufs=1)
        for pos, c in enumerate(GRPS[g]):
            nc.tensor.matmul(
                self.att[blk][:],
                p["gT_sb"][:, c * P:(c + 1) * P],
                ex[:, pos * 512:(pos + 1) * 512],
                start=c == 0, stop=c == NCH - 1)
        if g == NGRP - 1:
            self._emit_tail(blk)
            self.av_blk += 1

    def drain_av(self, emitted, all_=False):
        """Drain pending avs block-major; an av is eligible once its exp
        was emitted DEFER group-emissions ago AND its gT chunks' transposes
        have been emitted (emission order IS dependency order - a read
        emitted before its writer silently reads stale SBUF). At most 2
        per call so a backlog never opens a multi-us PE gap in the score
        stream."""
        n = 0
        while self.av_blk < NBLK and (all_ or n < 2):
            q = self.av_q[self.av_blk]
            if not q:
                break
            blk, g, ex = q[0]
            if GRPS[g][-1] >= self.gt_ready:
                break
            idx = self.emit_idx[(blk, g)]
            if not all_ and idx > emitted - DEFER:
                break
            q.popleft()
            self._emit_av(blk, g, ex)
            n += 1

    # ---- block tail: den-fold, recip, normalize, W, store --------------
    def _emit_tail(self, blk):
        nc, p = self.nc, self.p
        acc = self.accd[blk]
        den = p["ps"].tile([P, 512], F32, name=f"den{blk}", tag="tail",
                           bufs=1)
        if blk == NBLK - 1:
            # fold acc(g0..8) plus the raw ex tiles of groups 9/10
            srcs = [acc[:, s * 512:(s + 1) * 512] for s in range(3)]
            for g in (NGRP - 2, NGRP - 1):
                ex = self.exs[(blk, g)]
                srcs += [ex[:, s * 512:(s + 1) * 512]
                         for s in range(len(GRPS[g]))]
        else:
            srcs = [acc[:, s * 512:(s + 1) * 512] for s in range(3)]
        for s, src in enumerate(srcs):
            nc.tensor.matmul(den[:], p["onesb"][:], src,
                             start=s == 0, stop=s == len(srcs) - 1)
        recb = p["rec"].tile([P, 512], F32, name=f"recb{blk}", tag="recb",
                             bufs=2)
        nc.vector.reciprocal_approx_fast(out=recb[:], in_=den[:])
        attn = p["rec"].tile([P, 512], F32R, name=f"attn{blk}", tag="attn",
                             bufs=2)
        nc.vector.tensor_mul(attn[:], self.att[blk][:], recb[:])
        isl = slice(blk * 512, (blk + 1) * 512)
        for k in range(2):
            # k=1 borrows the att bank (free once normalize has read it)
            # so the two W projections don't serialize on the tail bank
            y = p["ps"].tile([P, 512], F32, name=f"y{blk}_{k}",
                             tag="tail" if k == 0 else "att", bufs=1)
            nc.tensor.matmul(y[:], p["wef_sb"][:, k * P:(k + 1) * P],
                             attn[:], start=True, stop=True)
            yo = p["rec"].tile([P, 512], F32, name=f"yo{blk}_{k}", tag="yo",
                               bufs=2)
            nc.vector.scalar_tensor_tensor(
                out=yo[:], in0=y[:], scalar=p["bfin_sb"][:, k:k + 1],
                in1=p["x_sb"][k][:, isl], op0=ADD, op1=ADD)
            nc.sync.dma_start(out=self.yout[k * P:(k + 1) * P, isl],
                              in_=yo[:])


def _kernel_body(ctx, tc, ins, yout):
    nc = tc.nc
    xin, thw, phw, gw, wef, tb, bfin = (
        ins["xin"], ins["thw"], ins["phw"], ins["gw"], ins["wef"],
        ins["tb"], ins["bfin"])

    consts = ctx.enter_context(tc.tile_pool(name="consts", bufs=1))
    big = ctx.enter_context(tc.tile_pool(name="big", bufs=1))

    # ---- dummies; exdum goes first on the ACT queue so walrus's
    # auto-inserted exp table load lands in the preamble window.
    dum_r = consts.tile([P, 512], F32R, name="dum_r")
    nc.vector.memset(dum_r.bitcast(F32)[:], 1.0)
    exdum = consts.tile([P, 1], F32, name="exdum")
    nc.scalar.activation(exdum[:], dum_r.bitcast(F32)[:, 0:1], AF.Exp)

    # ---- x load: 512-col slices, both HWDGE rings in parallel ----
    x_sb = [big.tile([P, N], F32R, name=f"x_sb{k}") for k in range(2)]

    # k=1 ring rides the GpSimd queue: the Scalar queue must stay
    # exp-only (a DMA descriptor issue costs ~650ns of ACT queue time
    # and serializes ahead of the exps).
    def xdma(t, k):
        tsl = slice(t * 512, (t + 1) * 512)
        eng = nc.sync if k == 0 else nc.gpsimd
        eng.dma_start(out=x_sb[k][:, tsl],
                      in_=_r(xin[k * P:(k + 1) * P, tsl]))

    for k in range(2):
        xdma(0, k)
    thw_sb = consts.tile([P, C], F32R, name="thw_sb")
    phw_sb = consts.tile([P, C], F32R, name="phw_sb")
    gw_sb = consts.tile([P, C], F32R, name="gw_sb")
    for k in range(2):
        nc.sync.dma_start(out=thw_sb[:, k * P:(k + 1) * P],
                          in_=_r(thw[k * P:(k + 1) * P, :]))
        nc.gpsimd.dma_start(out=phw_sb[:, k * P:(k + 1) * P],
                            in_=_r(phw[k * P:(k + 1) * P, :]))
        nc.sync.dma_start(out=gw_sb[:, k * P:(k + 1) * P],
                          in_=_r(gw[k * P:(k + 1) * P, :]))
    tb_sb = consts.tile([P, 1], F32, name="tb_sb")
    nc.gpsimd.dma_start(out=tb_sb[:], in_=tb[:, None])
    ident = consts.tile([P, P], F32, name="ident")
    nc.sync.dma_start(out=ident[:], in_=ins["ident"][:, :])
    for t in range(1, 8):
        for k in range(2):
            xdma(t, k)
    wef_sb = consts.tile([P, C], F32R, name="wef_sb")
    nc.sync.dma_start(out=wef_sb[:], in_=_r(wef[:, :]))
    bfin_sb = consts.tile([P, 2], F32, name="bfin_sb")
    nc.sync.dma_start(out=bfin_sb[:], in_=bfin.rearrange("(k p) -> p k", p=P))
    onesb = consts.tile([P, P], BF16, name="onesb")
    nc.vector.memset(onesb[:], 1.0)

    theta_sb = big.tile([P, HALF], F32R, name="theta_sb")
    phi_sb = big.tile([P, N], F32R, name="phi_sb")
    g_sb = big.tile([P, N], F32, name="g_sb")
    gT_sb = big.tile([P, N], BF16, name="gT_sb")

    # ---- PSUM: sc 2x[128,1536]=6 banks, att 1, tail 1 ----
    ps_pool = ctx.enter_context(tc.tile_pool(name="ps", bufs=1, space="PSUM"))
    pools = {
        "ps": ps_pool,
        "ex": ctx.enter_context(tc.tile_pool(name="ex", bufs=EXBUFS)),
        "rec": ctx.enter_context(tc.tile_pool(name="rec", bufs=2)),
        "theta_sb": theta_sb, "phi_sb": phi_sb, "gT_sb": gT_sb,
        "onesb": onesb, "wef_sb": wef_sb, "bfin_sb": bfin_sb, "x_sb": x_sb,
    }
    pipe = _Pipe(nc, pools, yout)

    # ---- HAM warmup while the first x slice streams in ----
    dum_ps = ps_pool.tile([P, 512], F32, name="dum_ps", tag="tail", bufs=1)
    for i in range(NWARM):
        nc.tensor.matmul(dum_ps[:], dum_r[:, 0:P], dum_r[:],
                         start=True, stop=True)

    # ---- per-slice projections batched into one sc-sized PSUM slot
    # (th | ph | g); g-transposes batch 4 chunks into the tail bank.
    def proj(t):
        tsl = slice(t * 512, (t + 1) * 512)
        slot = ps_pool.tile([P, 1536], F32, name=f"proj{t}", tag="sc",
                            bufs=2)
        if t < NBLK:
            for k in range(2):
                nc.tensor.matmul(slot[:, 0:512],
                                 thw_sb[:, k * P:(k + 1) * P],
                                 x_sb[k][:, tsl],
                                 start=k == 0, stop=k == 1)
        for k in range(2):
            nc.tensor.matmul(slot[:, 512:1024],
                             phw_sb[:, k * P:(k + 1) * P],
                             x_sb[k][:, tsl],
                             start=k == 0, stop=k == 1)
        for k in range(2):
            nc.tensor.matmul(slot[:, 1024:1536],
                             gw_sb[:, k * P:(k + 1) * P],
                             x_sb[k][:, tsl],
                             start=k == 0, stop=k == 1)
        # phi first: it is the copy that gates the next score groups
        nc.vector.tensor_copy(phi_sb[:, tsl], slot[:, 512:1024])
        if t < NBLK:
            nc.vector.tensor_scalar_add(theta_sb[:, tsl], slot[:, 0:512],
                                        tb_sb[:])
        nc.vector.tensor_copy(g_sb[:, tsl], slot[:, 1024:1536])

    def transposes(t):
        pst = ps_pool.tile([P, 512], F32, name=f"gt{t}", tag="tail", bufs=1)
        for j in range(4):
            jc = 4 * t + j
            nc.tensor.transpose(pst[:, j * P:(j + 1) * P],
                                g_sb[:, jc * P:(jc + 1) * P], ident[:])
        nc.vector.tensor_copy(gT_sb[:, 4 * t * P:(4 * t + 4) * P], pst[:])
        pipe.gt_ready = 4 * (t + 1)

    # group g needs phi slice (last_chunk // 4); block blk needs theta
    # slice blk. In the slice loop emit at most TWO ready groups per slice
    # (block-major priority) - the proj slot then rotates onto a slot
    # whose last reader is cheap, so projections track the DMA stream
    # instead of slipping to exp pace. The remaining groups bulk-emit
    # after the loop, block-major, paced by the sc slots.
    t_req = [GRPS[g][-1] // 4 for g in range(NGRP)]
    emitted = 0

    def emit_one(blk, g):
        nonlocal emitted
        pipe.emit_idx[(blk, g)] = emitted
        pipe.emit_group(blk, g)
        emitted += 1
        pipe.drain_av(emitted)

    # Projections run ONE SLICE AHEAD of their score groups: a proj's sc
    # slot predecessor (2 allocations back in the rotation) is then an
    # exp that is already done or in flight, so the per-slice dependency
    # cycle no longer contains a fresh exp and the loop runs ACT-paced.
    proj(0)
    proj(1)
    den0_next = 1
    for t in range(8):
        ready = [(blk, g) for blk in range(NBLK) for g in range(NGRP)
                 if (blk, g) not in pipe.emit_idx
                 and t_req[g] <= t and blk <= t + 1]
        for blk, g in sorted(ready)[:2]:
            emit_one(blk, g)
        if t + 2 < 8:
            proj(t + 2)
        if t >= 1:
            transposes(t - 1)
        # trickle block 0's deferred den chain behind its exps (max 2 per
        # step, each at least 2 emissions old) so tail(0) lands right
        # after block 0's last exp instead of ~10us later
        for _ in range(2):
            if (den0_next < NGRP and (0, den0_next) in pipe.emit_idx
                    and pipe.emit_idx[(0, den0_next)] <= emitted - 2):
                pipe.den_add(0, den0_next)
                den0_next += 1
            else:
                break
    transposes(7)
    for g in range(NGRP):
        if (0, g) not in pipe.emit_idx:
            emit_one(0, g)
    # rest of block 0's deferred denominator chain
    for g in range(den0_next, NGRP):
        pipe.den_add(0, g)
    rest = [(blk, g) for blk in range(NBLK) for g in range(NGRP)
            if (blk, g) not in pipe.emit_idx]
    for blk, g in sorted(rest):
        emit_one(blk, g)
    pipe.drain_av(emitted, all_=True)


_CACHE = {}


def _build():
    if "nc" in _CACHE:
        return _CACHE["nc"]
    nc = bacc.Bacc("TRN2", target_bir_lowering=False, debug=False,
                   enable_asserts=False, num_devices=1)
    ins = {
        "xin": nc.dram_tensor("xin", [C, N], F32, kind="ExternalInput").ap(),
        "thw": nc.dram_tensor("thw", [C, IC], F32, kind="ExternalInput").ap(),
        "phw": nc.dram_tensor("phw", [C, IC], F32, kind="ExternalInput").ap(),
        "gw": nc.dram_tensor("gw", [C, IC], F32, kind="ExternalInput").ap(),
        "wef": nc.dram_tensor("wef", [IC, C], F32, kind="ExternalInput").ap(),
        "tb": nc.dram_tensor("tb", [IC], F32, kind="ExternalInput").ap(),
        "bfin": nc.dram_tensor("bfin", [C], F32, kind="ExternalInput").ap(),
        "ident": nc.dram_tensor("ident", [P, P], F32,
                                kind="ExternalInput").ap(),
    }
    yout = nc.dram_tensor("yout", [C, HALF], F32, kind="ExternalOutput").ap()
    with tile.TileContext(nc) as tc:
        with ExitStack() as ctx:
            _kernel_body(ctx, tc, ins, yout)
    nc.compile()
    _CACHE["nc"] = nc
    return nc


def _host_prepare(inputs):
    """Host-side folds + per-core input maps."""
    ii = {k: np.ascontiguousarray(np.asarray(v, dtype=np.float32))
          for k, v in inputs.items()}
    inv = ii["bn_gamma"] / np.sqrt(ii["bn_var"] + BN_EPS)
    w_eff = ii["w_w"] * inv[:, None]                       # [C, IC]
    b_final = (w_eff @ ii["g_b"] + ii["w_b"] * inv
               + ii["bn_beta"] - ii["bn_mean"] * inv)      # [C]
    shared = {
        "thw": np.ascontiguousarray(ii["theta_w"].T),      # [C, IC]
        "phw": np.ascontiguousarray(ii["phi_w"].T),
        "gw": np.ascontiguousarray(ii["g_w"].T),
        "wef": np.ascontiguousarray(w_eff.T),              # [IC, C]
        "tb": ii["theta_b"],
        "bfin": np.ascontiguousarray(b_final),
        "ident": np.eye(P, dtype=np.float32),
    }
    x = ii["x"].reshape(B, C, N)
    in_maps = []
    for core in range(NCORES):
        b, h = divmod(core, 2)
        own = x[b][:, h * HALF:(h + 1) * HALF]
        oth = x[b][:, (1 - h) * HALF:(2 - h) * HALF]
        xin = np.ascontiguousarray(np.concatenate([own, oth], axis=1))
        in_maps.append({"xin": xin, **shared})
    return in_maps


def _gather(results, x_dtype):
    out = np.empty((B, C, N), dtype=np.float32)
    for core in range(NCORES):
        b, h = divmod(core, 2)
        out[b][:, h * HALF:(h + 1) * HALF] = results[core]["yout"]
    return out.reshape(B, C, H, W).astype(x_dtype, copy=False)


def kernel(**inputs):
    nc = _build()
    in_maps = _host_prepare(inputs)
    res = run_bass_kernel_spmd(nc, in_maps, core_ids=list(range(NCORES)))
    return _gather(res.results, np.asarray(inputs["x"]).dtype)
